# revision 61
# baseline (speedup 1.0000x reference)
"""Trainium2 Bass kernel for nn_MultiHeadedAttention (OpenNMT-style relative-position
multi-head attention with RoPE, causal mask, and relative key/value embeddings).

Sharding: tensor-parallel over heads — each of the 8 cores owns 2 of the 16 heads.
Each core computes q/k/v projections for its heads from the full input, runs
attention (with the relative-position score/value terms), and produces a partial
output projection (w_o restricted to its heads' columns). The host sums the 8
partial outputs (the "all-reduce" of the final linear, done on the host as part
of unsharding).

Key device-side tricks:
 - Head dim is deinterleaved (evens then odds) via host-side weight row permutation
   so RoPE pair rotation becomes two partition-block multiplies plus an SBUF
   partition-swap DMA (no cross-lane ops).
 - The relative-position score term R[q, j] = q·rel_emb[j] collapses to a banded
   add: scores[q,k] += R[q, clip(k-q+32)]. Per-row R0 = R[q,0] is constant per row
   and cancels in softmax, so only the band (R[q,j]-R0 for j in 0..32, and -1e18
   for the causal-masked j>32) is materialized, stored in a DRAM row-padded buffer
   and read back with a diagonal (row-stride W-1) access pattern. This also
   implements the causal mask for free.
 - The relative-position value term attn@rel collapses to A_band@rel_emb where
   A_band[q,j] = attn[q, q+j-31]; attn's diagonal band is written to DRAM with a
   diagonal access pattern and read back transposed; the j=0 (clipped) bucket is
   1 - sum(band) since softmax rows sum to 1.
"""
import sys

sys.path.insert(0, '/opt/trn_rl_repo')

import numpy as np
import ml_dtypes

import concourse.bass as bass
import concourse.mybir as mybir
import concourse.tile as tile
from concourse import bacc
from concourse.bass_utils import run_bass_kernel_spmd

F32 = mybir.dt.float32
BF16 = mybir.dt.bfloat16

HEADS = 16
D = 64                 # dim per head
HC = 2                 # heads per core
B = 2
L = 1024
MD = 1024              # model dim
T = B * L              # total tokens
NCORES = 8
MAXREL = 32
NEG = -1e18

RW = 288               # R_pad row width: [0,127)=0, [127,160)=band, [160,288)=-1e18
AW = 192               # attn band buffer row width

ROPE_BASE = 10000.0


def _rope_tables():
    inv_freq = 1.0 / (ROPE_BASE ** (np.arange(0, D, 2, dtype=np.float64) / D))  # [32]
    t = np.arange(L, dtype=np.float64)
    ang = np.outer(t, inv_freq)  # [L, 32]
    cos = np.cos(ang).astype(np.float32)  # [L, 32]
    sin = np.sin(ang).astype(np.float32)
    cos_rep = np.zeros((128, L), dtype=np.float32)
    sin_sgn = np.zeros((128, L), dtype=np.float32)
    for r in range(128):
        i = r % 32
        sign = -1.0 if (r % 64) < 32 else 1.0
        cos_rep[r] = cos[:, i]
        sin_sgn[r] = sign * sin[:, i]
    return cos_rep, sin_sgn


def build_nc():
    """Build the per-core SPMD program (identical on all cores; inputs differ)."""
    nc = bacc.Bacc()

    xT = nc.dram_tensor("xT", [MD, T], BF16, kind="ExternalInput")
    wqT = nc.dram_tensor("wqT", [MD, 128], BF16, kind="ExternalInput")
    wkT = nc.dram_tensor("wkT", [MD, 128], BF16, kind="ExternalInput")
    wvT = nc.dram_tensor("wvT", [MD, 128], BF16, kind="ExternalInput")
    woT = nc.dram_tensor("woT", [128, MD], BF16, kind="ExternalInput")
    relT33 = nc.dram_tensor("relT33", [64, 33], BF16, kind="ExternalInput")
    relvp = nc.dram_tensor("relvp", [65, 64], BF16, kind="ExternalInput")
    outT = nc.dram_tensor("outT", [MD, T], F32, kind="ExternalOutput")

    # DRAM scratch, one buffer per (batch, head) stream
    rpads = [nc.dram_tensor(f"rpad{i}", [L * RW], BF16) for i in range(4)]
    abufs = [nc.dram_tensor(f"abuf{i}", [L * AW], BF16) for i in range(4)]

    cos_np, sin_np = _rope_tables()
    cos_c = nc.inline_tensor(cos_np, name="cosrep")
    sin_c = nc.inline_tensor(sin_np, name="sinsgn")
    ident_c = nc.inline_tensor(np.eye(128, dtype=ml_dtypes.bfloat16), name="identbf")

    with tile.TileContext(nc) as tc:
        with (
            tc.tile_pool(name="cst", bufs=1) as cst,
            tc.tile_pool(name="big", bufs=1) as big,
        ):
            # ---------- constants ----------
            cos_sb = cst.tile([128, L], F32)
            sin_sb = cst.tile([128, L], F32)
            ident = cst.tile([128, 128], BF16)
            nc.scalar.dma_start(out=cos_sb, in_=cos_c.ap())
            nc.scalar.dma_start(out=sin_sb, in_=sin_c.ap())
            nc.scalar.dma_start(out=ident, in_=ident_c.ap())

            relT_rep = cst.tile([128, 33], BF16)
            nc.scalar.dma_start(out=relT_rep[0:64, :], in_=relT33[:, :])
            nc.scalar.dma_start(out=relT_rep[64:128, :], in_=relT33[:, :])
            relvp_sb = cst.tile([65, 64], BF16)
            nc.scalar.dma_start(out=relvp_sb, in_=relvp[:, :])

            # ---------- load weights + xT first (sync queue), init scratch on
            # the scalar queue so projections start ASAP ----------
            wo_sb = cst.tile([128, MD], BF16)
            nc.sync.dma_start(out=wo_sb, in_=woT[:, :])

            # one contiguous descriptor per 128-row block: full-width row images
            rin = cst.tile([128, RW], BF16)
            nc.vector.memset(rin, 0.0)
            nc.vector.memset(rin[:, 160:288], NEG)
            zin_a = cst.tile([128, AW], BF16)
            nc.vector.memset(zin_a, 0.0)
            for buf in rpads:
                v = buf.ap().rearrange("(q c) -> q c", c=RW)
                for rb in range(8):
                    nc.gpsimd.dma_start(out=v[rb * 128:(rb + 1) * 128, :], in_=rin)
            for buf in abufs:
                v = buf.ap().rearrange("(q c) -> q c", c=AW)
                for rb in range(8):
                    nc.gpsimd.dma_start(out=v[rb * 128:(rb + 1) * 128, :], in_=zin_a)

            # ---------- projections ----------
            qT_sb = big.tile([128, T], BF16)
            kT_sb = big.tile([128, T], BF16)
            v_sb = big.tile([128, 16 * 128], BF16)   # [k-token-part, token-tile, feat]

            with (
                tc.tile_pool(name="psp", bufs=4, space="PSUM") as psp,
                tc.tile_pool(name="psv", bufs=2, space="PSUM") as psv,
                tc.tile_pool(name="rope", bufs=1) as rope_pool,
            ):
                w_sbs = {}
                for name, wt in (("q", wqT), ("k", wkT), ("v", wvT)):
                    w_sb = rope_pool.tile([128, 8 * 128], BF16, tag=f"w{name}")
                    # one DMA: src iterated (p, m, f) matching [p, m*128+f] layout
                    nc.sync.dma_start(out=w_sb, in_=bass.AP(
                        tensor=wt.ap().tensor, offset=0,
                        ap=[[128, 128], [128 * 128, 8], [1, 128]]))
                    w_sbs[name] = w_sb
                xT_sb = rope_pool.tile([128, 8 * T], BF16)  # chunk m at [m*T,(m+1)*T)
                for m in range(8):
                    eng = nc.sync if m % 2 == 0 else nc.scalar
                    eng.dma_start(out=xT_sb[:, m * T:(m + 1) * T],
                                  in_=xT[m * 128:(m + 1) * 128, :])
                qraw = rope_pool.tile([128, T], F32)
                kraw = rope_pool.tile([128, T], F32)
                vTs = rope_pool.tile([128, T], BF16)
                # q projection with the m (xT-chunk) loop OUTER so PE work starts
                # as soon as each xT chunk lands, overlapping the initial load
                qps0 = psp.tile([128, 512], F32, tag="proj")
                qps1 = psp.tile([128, 512], F32, tag="proj")
                qps2 = psp.tile([128, 512], F32, tag="proj")
                qps3 = psp.tile([128, 512], F32, tag="proj")
                qps = [qps0, qps1, qps2, qps3]
                for m in range(8):
                    for tch in range(4):
                        nc.tensor.matmul(
                            qps[tch],
                            lhsT=w_sbs["q"][:, m * 128:(m + 1) * 128],
                            rhs=xT_sb[:, m * T + tch * 512: m * T + tch * 512 + 512],
                            start=(m == 0), stop=(m == 7))
                for tch in range(4):
                    nc.scalar.copy(out=qraw[:, tch * 512:(tch + 1) * 512], in_=qps[tch])
                for name, raw in (("k", kraw), ("v", vTs)):
                    w_sb = w_sbs[name]
                    for tch in range(4):
                        ps = psp.tile([128, 512], F32, tag="proj")
                        for m in range(8):
                            nc.tensor.matmul(
                                ps,
                                lhsT=w_sb[:, m * 128:(m + 1) * 128],
                                rhs=xT_sb[:, m * T + tch * 512: m * T + tch * 512 + 512],
                                start=(m == 0), stop=(m == 7))
                        nc.scalar.copy(out=raw[:, tch * 512:(tch + 1) * 512], in_=ps)

                # RoPE on q/k raw (f32) -> bf16 qT_sb/kT_sb; the sin-side multiply
                # runs on the otherwise-idle GpSimd engine
                shuf = rope_pool.tile([128, T], F32)
                m1 = rope_pool.tile([128, T], F32)
                m2 = rope_pool.tile([128, T], F32)
                for raw, dst in ((qraw, qT_sb), (kraw, kT_sb)):
                    for bs in (0, 64):
                        nc.sync.dma_start(out=shuf[bs:bs + 32, :], in_=raw[bs + 32:bs + 64, :])
                        nc.sync.dma_start(out=shuf[bs + 32:bs + 64, :], in_=raw[bs:bs + 32, :])
                    for h2 in range(2):
                        cs = slice(h2 * L, (h2 + 1) * L)
                        nc.vector.tensor_mul(m1[:, cs], raw[:, cs], cos_sb)
                        nc.vector.tensor_mul(m2[:, cs], shuf[:, cs], sin_sb)
                    nc.vector.tensor_add(dst, m1, m2)

                # v: transpose [feat, tok] -> [tok, feat] tiles
                for tt in range(16):
                    pt = psv.tile([128, 128], BF16, tag="vtr")
                    nc.tensor.transpose(pt, vTs[:, tt * 128:(tt + 1) * 128], ident)
                    nc.scalar.copy(out=v_sb[:, tt * 128:(tt + 1) * 128], in_=pt)

            # ---------- attention ----------
            ctxT_sb = big.tile([128, T], BF16)

            with (
                tc.tile_pool(name="scps", bufs=3, space="PSUM") as scps,
                tc.tile_pool(name="psm", bufs=3, space="PSUM") as psm,
                tc.tile_pool(name="psc", bufs=2, space="PSUM") as psc,
                tc.tile_pool(name="sc", bufs=2) as sc_pool,
                tc.tile_pool(name="at", bufs=2) as at_pool,
                tc.tile_pool(name="atT", bufs=4) as atT_pool,
                tc.tile_pool(name="sml", bufs=16) as sml,
                tc.tile_pool(name="gth", bufs=3) as gth,
            ):
                def out_proj(tch, oc0, oc1):
                    # partial w_o for one 512-token chunk (cols tch*512..);
                    # all 8 output-row blocks gathered into one SBUF image and
                    # written with a single 3D-AP DMA
                    osb = sc_pool.tile([128, 8 * 512], F32, tag="osb")
                    for oc in range(oc0, oc1):
                        po = scps.tile([128, 512], F32, tag="sc")
                        nc.tensor.matmul(po, lhsT=wo_sb[:, oc * 128:(oc + 1) * 128],
                                         rhs=ctxT_sb[:, tch * 512:(tch + 1) * 512],
                                         start=True, stop=True)
                        if oc % 2:
                            nc.scalar.copy(out=osb[:, oc * 512:(oc + 1) * 512], in_=po)
                        else:
                            nc.vector.tensor_copy(osb[:, oc * 512:(oc + 1) * 512], po)
                    nc.sync.dma_start(
                        out=bass.AP(tensor=outT.ap().tensor, offset=tch * 512,
                                    ap=[[T, 128], [128 * T, 8], [1, 512]]),
                        in_=osb)

                def pieces_of(tq):
                    """Score segments for q-tile tq: <=512-wide matmul pieces.
                    Returns (s, e) pairs; the relative band [kf, K) is added to
                    whichever piece(s) cover it."""
                    K = tq * 128 + 128
                    return [(s, min(K, s + 512)) for s in range(0, K, 512)]

                def emit_phaseA(b, hh, rp):
                    st = 2 * b + hh
                    base = 64 * hh
                    rp_v = rp.ap().rearrange("(q c) -> q c", c=RW)
                    Rbig = gth.tile([128, 8 * 33], BF16, tag=f"rbig{st}")
                    for tq in range(8):
                        q0 = tq * 128
                        lq = qT_sb[base:base + 64, b * L + q0: b * L + q0 + 128]
                        pR = psm.tile([128, 512], F32, tag="sm")
                        nc.tensor.matmul(pR[:, 0:33], lhsT=lq,
                                         rhs=relT_rep[base:base + 64, :],
                                         start=True, stop=True)
                        R0 = sml.tile([128, 1], F32, tag="r0")
                        nc.scalar.copy(out=R0, in_=pR[:, 0:1])
                        nc.vector.tensor_scalar_sub(Rbig[:, tq * 33:(tq + 1) * 33],
                                                    in0=pR[:, 0:33], scalar1=R0)
                    nc.scalar.dma_start(
                        out=bass.AP(tensor=rp.ap().tensor, offset=127,
                                    ap=[[RW, 128], [RW * 128, 8], [1, 33]]),
                        in_=Rbig)

                def emit_tile(b, hh, rp, ab, a, tq, attnT, ATd2_sb):
                    st = 2 * b + hh
                    base = 64 * hh
                    q0 = tq * 128
                    K = q0 + 128
                    lq = qT_sb[base:base + 64, b * L + q0: b * L + q0 + 128]
                    attn = at_pool.tile([128, 1024], BF16, tag=f"at{st}")
                    kf = max(0, q0 - 32)
                    # band gather once per tile
                    wb = K - kf
                    rbg = gth.tile([128, 160], BF16, tag=f"rbg{st}")
                    diag = bass.AP(tensor=rp.ap().tensor,
                                   offset=q0 * RW + 159 + (kf - q0),
                                   ap=[[RW - 1, 128], [1, wb]])
                    nc.scalar.dma_start(out=rbg[:, :wb], in_=diag)
                    dens = []
                    for (s, e) in pieces_of(tq):
                        ps = scps.tile([128, 512], F32, tag="sc")
                        nc.tensor.matmul(
                            ps[:, :e - s], lhsT=lq,
                            rhs=kT_sb[base:base + 64, b * L + s: b * L + e],
                            start=True, stop=True)
                        # add the part of the relative band overlapping this piece
                        lo, hi = max(s, kf), e
                        if lo < hi:
                            nc.vector.tensor_add(ps[:, lo - s:hi - s],
                                                 ps[:, lo - s:hi - s],
                                                 rbg[:, lo - kf:hi - kf])
                        # softmax without max-subtraction: post-scale scores are
                        # bounded (|s|/8 < ~4); masked lanes are -1e18 -> exp = 0.
                        den = sml.tile([128, 1], F32, tag="dn")
                        nc.scalar.activation(out=attn[:, s:e], in_=ps[:, :e - s],
                                             func=mybir.ActivationFunctionType.Exp,
                                             bias=0.0, scale=0.125, accum_out=den)
                        dens.append(den)
                    # unnormalized attn band -> ab (diagonal store), ASAP
                    k0a = max(0, q0 - 31)
                    wa = q0 + 128 - k0a
                    abdst = bass.AP(tensor=ab.ap().tensor,
                                    offset=q0 * AW + 31 + (k0a - q0),
                                    ap=[[AW - 1, 128], [1, wa]])
                    nc.sync.dma_start(out=abdst, in_=attn[:, k0a:k0a + wa])
                    if len(dens) > 1:
                        dtot = sml.tile([128, 1], F32, tag="dn")
                        nc.vector.tensor_add(dtot, dens[0], dens[1])
                    else:
                        dtot = dens[0]
                    rden = sml.tile([128, 1], F32, tag="rd")
                    nc.vector.reciprocal(rden, dtot)
                    # normalization folded into the transpose matmuls: rhs is
                    # diag(1/den) instead of the identity
                    dg = gth.tile([128, 128], BF16, tag=f"dg{st}")
                    nc.vector.tensor_scalar_mul(dg, in0=ident, scalar1=rden)
                    half = (tq - a) * 128
                    for kt in range(tq + 1):
                        ptr = psm.tile([128, 512], F32, tag="sm")
                        nc.tensor.matmul(ptr[:, 0:128],
                                         lhsT=attn[:, kt * 128:(kt + 1) * 128],
                                         rhs=dg, start=True, stop=True)
                        nc.vector.tensor_copy(
                            attnT[:, kt * 256 + half: kt * 256 + half + 128],
                            ptr[:, 0:128])
                    # attn diag band back (rows of ab, contiguous inner):
                    # Ad2[:, 0:64] = attn_u[q, q+j'-31]; col 64 = den - rowsum;
                    # then the whole [128, 65] row-block is scaled by 1/den.
                    Ad2 = gth.tile([128, 65], BF16, tag=f"ad2{st}")
                    nc.sync.dma_start(out=Ad2[:, 0:64], in_=bass.AP(
                        tensor=ab.ap().tensor, offset=q0 * AW,
                        ap=[[AW, 128], [1, 64]]))
                    Ssum = sml.tile([128, 1], F32, tag="ss")
                    nc.vector.reduce_sum(Ssum, Ad2[:, 0:64], axis=mybir.AxisListType.X)
                    nc.vector.tensor_sub(Ad2[:, 64:65], dtot, Ssum)
                    nc.vector.tensor_scalar_mul(Ad2, in0=Ad2, scalar1=rden)
                    ptr2 = psm.tile([128, 512], BF16, tag="sm")
                    nc.tensor.transpose(ptr2[0:65, 0:128], Ad2, ident)
                    nc.vector.tensor_copy(ATd2_sb[:, half:half + 128],
                                          ptr2[0:65, 0:128])

                def emit_pair(b, hh, rp, ab, a):
                    st = 2 * b + hh
                    base = 64 * hh
                    attnT = atT_pool.tile([128, 8 * 256], BF16)
                    ATd2_sb = gth.tile([65, 256], BF16, tag=f"atd{st}")
                    for tq in (a, a + 1):
                        emit_tile(b, hh, rp, ab, a, tq, attnT, ATd2_sb)
                    pctx = psc.tile([128, 256], F32)
                    rows = slice(base, base + 64)
                    for kt in range(a + 1):
                        nc.tensor.matmul(
                            pctx[rows, :],
                            lhsT=v_sb[:, (8 * b + kt) * 128 + base:
                                      (8 * b + kt) * 128 + base + 64],
                            rhs=attnT[:, kt * 256:(kt + 1) * 256],
                            start=(kt == 0), stop=False)
                    # k-tile a+1 only feeds q-tile a+1 (right half); skip the zero half
                    nc.tensor.matmul(
                        pctx[rows, 128:256],
                        lhsT=v_sb[:, (8 * b + a + 1) * 128 + base:
                                  (8 * b + a + 1) * 128 + base + 64],
                        rhs=attnT[:, (a + 1) * 256 + 128:(a + 2) * 256],
                        start=False, stop=False)
                    nc.tensor.matmul(pctx[rows, :], lhsT=relvp_sb, rhs=ATd2_sb,
                                     start=False, stop=True)
                    nc.scalar.copy(out=ctxT_sb[rows, b * L + a * 128: b * L + a * 128 + 256],
                                   in_=pctx[rows, :])

                # all four (batch, head) streams interleaved to keep the PE
                # dense; output projection of each finished 512-token column
                # range interleaved to fill PE gaps
                for b in range(2):
                    for hh in (0, 1):
                        emit_phaseA(b, hh, rpads[2 * b + hh])
                for a in (0, 2, 4, 6):
                    for b in range(2):
                        for hh in (0, 1):
                            emit_pair(b, hh, rpads[2 * b + hh], abufs[2 * b + hh], a)
                    if a == 2:
                        out_proj(0, 0, 4)
                        out_proj(2, 0, 4)
                    elif a == 4:
                        out_proj(0, 4, 8)
                        out_proj(2, 4, 8)
                out_proj(1, 0, 8)
                out_proj(3, 0, 8)
    nc.compile()
    return nc


_PERM64 = np.concatenate([np.arange(0, 64, 2), np.arange(1, 64, 2)])


def prep_inputs(x, w_q, w_k, w_v, w_o, rel_emb):
    """Host-side sharding/layout prep. Returns per-core input maps."""
    bf = ml_dtypes.bfloat16
    xT = np.ascontiguousarray(x.reshape(T, MD).T).astype(bf)
    relT33 = np.ascontiguousarray(rel_emb.T[_PERM64][:, 0:33]).astype(bf)
    permv = np.concatenate([np.arange(1, 65), [0]])
    relvp = np.ascontiguousarray(rel_emb[permv, :]).astype(bf)
    in_maps = []
    for c in range(NCORES):
        rows = np.arange(128 * c, 128 * (c + 1))
        rows_d = np.concatenate([128 * c + 64 * h + _PERM64 for h in range(HC)])
        in_maps.append({
            "xT": xT,
            "wqT": np.ascontiguousarray(w_q[rows_d, :].T).astype(bf),
            "wkT": np.ascontiguousarray(w_k[rows_d, :].T).astype(bf),
            "wvT": np.ascontiguousarray(w_v[rows, :].T).astype(bf),
            "woT": np.ascontiguousarray(w_o[:, rows].T).astype(bf),
            "relT33": relT33,
            "relvp": relvp,
        })
    return in_maps


_NC_CACHE = None


def get_nc():
    global _NC_CACHE
    if _NC_CACHE is None:
        _NC_CACHE = build_nc()
    return _NC_CACHE


def kernel(x, w_q, w_k, w_v, w_o, rel_emb):
    x = np.asarray(x, dtype=np.float32)
    in_maps = prep_inputs(np.asarray(x, np.float32), np.asarray(w_q, np.float32),
                          np.asarray(w_k, np.float32), np.asarray(w_v, np.float32),
                          np.asarray(w_o, np.float32), np.asarray(rel_emb, np.float32))
    nc = get_nc()
    res = run_bass_kernel_spmd(nc, in_maps, list(range(NCORES)))
    acc = np.zeros((MD, T), dtype=np.float32)
    for r in res.results:
        acc += r["outT"]
    return np.ascontiguousarray(acc.T).reshape(B, L, MD)


# revision 62
# speedup vs baseline: 1.0600x; 1.0600x over previous
"""Trainium2 Bass kernel for nn_MultiHeadedAttention (OpenNMT-style relative-position
multi-head attention with RoPE, causal mask, and relative key/value embeddings).

Sharding: tensor-parallel over heads — each of the 8 cores owns 2 of the 16 heads.
Each core computes q/k/v projections for its heads from the full input, runs
attention (with the relative-position score/value terms), and produces a partial
output projection (w_o restricted to its heads' columns). The host sums the 8
partial outputs (the "all-reduce" of the final linear, done on the host as part
of unsharding).

Key device-side tricks:
 - Head dim is deinterleaved (evens then odds) via host-side weight row permutation
   so RoPE pair rotation becomes two partition-block multiplies plus an SBUF
   partition-swap DMA (no cross-lane ops).
 - The relative-position score term R[q, j] = q·rel_emb[j] collapses to a banded
   add: scores[q,k] += R[q, clip(k-q+32)]. Per-row R0 = R[q,0] is constant per row
   and cancels in softmax, so only the band (R[q,j]-R0 for j in 0..32, and -1e18
   for the causal-masked j>32) is materialized, stored in a DRAM row-padded buffer
   and read back with a diagonal (row-stride W-1) access pattern. This also
   implements the causal mask for free.
 - The relative-position value term attn@rel collapses to A_band@rel_emb where
   A_band[q,j] = attn[q, q+j-31]; attn's diagonal band is written to DRAM with a
   diagonal access pattern and read back transposed; the j=0 (clipped) bucket is
   1 - sum(band) since softmax rows sum to 1.
"""
import sys

sys.path.insert(0, '/opt/trn_rl_repo')

import numpy as np
import ml_dtypes

import concourse.bass as bass
import concourse.mybir as mybir
import concourse.tile as tile
from concourse import bacc
from concourse.bass_utils import run_bass_kernel_spmd

F32 = mybir.dt.float32
BF16 = mybir.dt.bfloat16

HEADS = 16
D = 64                 # dim per head
HC = 2                 # heads per core
B = 2
L = 1024
MD = 1024              # model dim
T = B * L              # total tokens
NCORES = 8
MAXREL = 32
NEG = -1e18

RW = 288               # R_pad row width: [0,127)=0, [127,160)=band, [160,288)=-1e18
AW = 192               # attn band buffer row width

ROPE_BASE = 10000.0


def _rope_tables():
    inv_freq = 1.0 / (ROPE_BASE ** (np.arange(0, D, 2, dtype=np.float64) / D))  # [32]
    t = np.arange(L, dtype=np.float64)
    ang = np.outer(t, inv_freq)  # [L, 32]
    cos = np.cos(ang).astype(np.float32)  # [L, 32]
    sin = np.sin(ang).astype(np.float32)
    cos_rep = np.zeros((128, L), dtype=np.float32)
    sin_sgn = np.zeros((128, L), dtype=np.float32)
    for r in range(128):
        i = r % 32
        sign = -1.0 if (r % 64) < 32 else 1.0
        cos_rep[r] = cos[:, i]
        sin_sgn[r] = sign * sin[:, i]
    return cos_rep, sin_sgn


def build_nc():
    """Build the per-core SPMD program (identical on all cores; inputs differ)."""
    nc = bacc.Bacc()

    xT = nc.dram_tensor("xT", [MD, T], BF16, kind="ExternalInput")
    wqT = nc.dram_tensor("wqT", [MD, 128], BF16, kind="ExternalInput")
    wkT = nc.dram_tensor("wkT", [MD, 128], BF16, kind="ExternalInput")
    wvT = nc.dram_tensor("wvT", [MD, 128], BF16, kind="ExternalInput")
    woT = nc.dram_tensor("woT", [128, MD], BF16, kind="ExternalInput")
    relT33 = nc.dram_tensor("relT33", [64, 33], BF16, kind="ExternalInput")
    relvp = nc.dram_tensor("relvp", [65, 64], BF16, kind="ExternalInput")
    outT = nc.dram_tensor("outT", [MD, T], F32, kind="ExternalOutput")

    # DRAM scratch, one buffer per (batch, head) stream
    rpads = [nc.dram_tensor(f"rpad{i}", [L * RW], BF16) for i in range(4)]
    abufs = [nc.dram_tensor(f"abuf{i}", [L * AW], BF16) for i in range(4)]

    cos_np, sin_np = _rope_tables()
    cos_c = nc.inline_tensor(cos_np, name="cosrep")
    sin_c = nc.inline_tensor(sin_np, name="sinsgn")
    ident_c = nc.inline_tensor(np.eye(128, dtype=ml_dtypes.bfloat16), name="identbf")

    with tile.TileContext(nc) as tc:
        with (
            tc.tile_pool(name="cst", bufs=1) as cst,
            tc.tile_pool(name="big", bufs=1) as big,
        ):
            # ---------- constants ----------
            cos_sb = cst.tile([128, L], F32)
            sin_sb = cst.tile([128, L], F32)
            ident = cst.tile([128, 128], BF16)
            nc.scalar.dma_start(out=cos_sb, in_=cos_c.ap())
            nc.scalar.dma_start(out=sin_sb, in_=sin_c.ap())
            nc.scalar.dma_start(out=ident, in_=ident_c.ap())

            relT_rep = cst.tile([128, 33], BF16)
            nc.scalar.dma_start(out=relT_rep[0:64, :], in_=relT33[:, :])
            nc.scalar.dma_start(out=relT_rep[64:128, :], in_=relT33[:, :])
            relvp_sb = cst.tile([65, 64], BF16)
            nc.scalar.dma_start(out=relvp_sb, in_=relvp[:, :])

            # ---------- load weights + xT first (sync queue), init scratch on
            # the scalar queue so projections start ASAP ----------
            wo_sb = cst.tile([128, MD], BF16)
            nc.sync.dma_start(out=wo_sb, in_=woT[:, :])

            # one contiguous descriptor per 128-row block: full-width row images
            rin = cst.tile([128, RW], BF16)
            nc.vector.memset(rin, 0.0)
            nc.vector.memset(rin[:, 160:288], NEG)
            zin_a = cst.tile([128, AW], BF16)
            nc.vector.memset(zin_a, 0.0)
            for buf in rpads:
                v = buf.ap().rearrange("(q c) -> q c", c=RW)
                for rb in range(8):
                    nc.gpsimd.dma_start(out=v[rb * 128:(rb + 1) * 128, :], in_=rin)
            for buf in abufs:
                v = buf.ap().rearrange("(q c) -> q c", c=AW)
                for rb in range(8):
                    nc.gpsimd.dma_start(out=v[rb * 128:(rb + 1) * 128, :], in_=zin_a)

            # ---------- projections ----------
            qT_sb = big.tile([128, T], BF16)
            kT_sb = big.tile([128, T], BF16)
            v_sb = big.tile([128, 16 * 128], BF16)   # [k-token-part, token-tile, feat]

            with (
                tc.tile_pool(name="psp", bufs=4, space="PSUM") as psp,
                tc.tile_pool(name="rope", bufs=1) as rope_pool,
            ):
                w_sbs = {}
                for name, wt in (("q", wqT), ("k", wkT), ("v", wvT)):
                    w_sb = rope_pool.tile([128, 8 * 128], BF16, tag=f"w{name}")
                    # one DMA: src iterated (p, m, f) matching [p, m*128+f] layout
                    nc.sync.dma_start(out=w_sb, in_=bass.AP(
                        tensor=wt.ap().tensor, offset=0,
                        ap=[[128, 128], [128 * 128, 8], [1, 128]]))
                    w_sbs[name] = w_sb
                xT_sb = rope_pool.tile([128, 8 * T], BF16)  # chunk m at [m*T,(m+1)*T)
                for m in range(8):
                    eng = nc.sync if m % 2 == 0 else nc.scalar
                    eng.dma_start(out=xT_sb[:, m * T:(m + 1) * T],
                                  in_=xT[m * 128:(m + 1) * 128, :])
                qraw = rope_pool.tile([128, T], F32)
                kraw = rope_pool.tile([128, T], F32)
                vTs = rope_pool.tile([128, T], BF16)
                # q projection with the m (xT-chunk) loop OUTER so PE work starts
                # as soon as each xT chunk lands, overlapping the initial load
                qps0 = psp.tile([128, 512], F32, tag="proj")
                qps1 = psp.tile([128, 512], F32, tag="proj")
                qps2 = psp.tile([128, 512], F32, tag="proj")
                qps3 = psp.tile([128, 512], F32, tag="proj")
                qps = [qps0, qps1, qps2, qps3]
                for m in range(8):
                    for tch in range(4):
                        nc.tensor.matmul(
                            qps[tch],
                            lhsT=w_sbs["q"][:, m * 128:(m + 1) * 128],
                            rhs=xT_sb[:, m * T + tch * 512: m * T + tch * 512 + 512],
                            start=(m == 0), stop=(m == 7))
                for tch in range(4):
                    nc.scalar.copy(out=qraw[:, tch * 512:(tch + 1) * 512], in_=qps[tch])
                for name, raw in (("k", kraw), ("v", vTs)):
                    w_sb = w_sbs[name]
                    for tch in range(4):
                        ps = psp.tile([128, 512], F32, tag="proj")
                        for m in range(8):
                            nc.tensor.matmul(
                                ps,
                                lhsT=w_sb[:, m * 128:(m + 1) * 128],
                                rhs=xT_sb[:, m * T + tch * 512: m * T + tch * 512 + 512],
                                start=(m == 0), stop=(m == 7))
                        nc.scalar.copy(out=raw[:, tch * 512:(tch + 1) * 512], in_=ps)

                # RoPE on q/k raw (f32) -> bf16 qT_sb/kT_sb; the sin-side multiply
                # runs on the otherwise-idle GpSimd engine
                shuf = rope_pool.tile([128, T], F32)
                m1 = rope_pool.tile([128, T], F32)
                m2 = rope_pool.tile([128, T], F32)
                for raw, dst in ((qraw, qT_sb), (kraw, kT_sb)):
                    for bs in (0, 64):
                        nc.sync.dma_start(out=shuf[bs:bs + 32, :], in_=raw[bs + 32:bs + 64, :])
                        nc.sync.dma_start(out=shuf[bs + 32:bs + 64, :], in_=raw[bs:bs + 32, :])
                    for h2 in range(2):
                        cs = slice(h2 * L, (h2 + 1) * L)
                        nc.vector.tensor_mul(m1[:, cs], raw[:, cs], cos_sb)
                        nc.vector.tensor_mul(m2[:, cs], shuf[:, cs], sin_sb)
                    nc.vector.tensor_add(dst, m1, m2)

                # v: transpose [feat, tok] -> [tok, feat] tiles
                for tt in range(16):
                    pt = psp.tile([128, 128], BF16, tag="vtr")
                    nc.tensor.transpose(pt, vTs[:, tt * 128:(tt + 1) * 128], ident)
                    nc.scalar.copy(out=v_sb[:, tt * 128:(tt + 1) * 128], in_=pt)

            # ---------- attention ----------
            ctxT_sb = big.tile([128, T], BF16)

            with (
                tc.tile_pool(name="scps", bufs=3, space="PSUM") as scps,
                tc.tile_pool(name="psm", bufs=3, space="PSUM") as psm,
                tc.tile_pool(name="psc", bufs=2, space="PSUM") as psc,
                tc.tile_pool(name="sc", bufs=2) as sc_pool,
                tc.tile_pool(name="at", bufs=2) as at_pool,
                tc.tile_pool(name="atT", bufs=4) as atT_pool,
                tc.tile_pool(name="sml", bufs=16) as sml,
                tc.tile_pool(name="gth", bufs=3) as gth,
            ):
                def out_proj(tch, oc0, oc1):
                    # partial w_o for one 512-token chunk (cols tch*512..);
                    # all 8 output-row blocks gathered into one SBUF image and
                    # written with a single 3D-AP DMA
                    osb = sc_pool.tile([128, 8 * 512], F32, tag="osb")
                    for oc in range(oc0, oc1):
                        po = scps.tile([128, 512], F32, tag="sc")
                        nc.tensor.matmul(po, lhsT=wo_sb[:, oc * 128:(oc + 1) * 128],
                                         rhs=ctxT_sb[:, tch * 512:(tch + 1) * 512],
                                         start=True, stop=True)
                        if oc % 2:
                            nc.scalar.copy(out=osb[:, oc * 512:(oc + 1) * 512], in_=po)
                        else:
                            nc.vector.tensor_copy(osb[:, oc * 512:(oc + 1) * 512], po)
                    nc.sync.dma_start(
                        out=bass.AP(tensor=outT.ap().tensor, offset=tch * 512,
                                    ap=[[T, 128], [128 * T, 8], [1, 512]]),
                        in_=osb)

                def pieces_of(tq):
                    """Score segments for q-tile tq: <=512-wide matmul pieces.
                    Returns (s, e) pairs; the relative band [kf, K) is added to
                    whichever piece(s) cover it."""
                    K = tq * 128 + 128
                    return [(s, min(K, s + 512)) for s in range(0, K, 512)]

                def emit_phaseA(b, hh, rp):
                    st = 2 * b + hh
                    base = 64 * hh
                    rp_v = rp.ap().rearrange("(q c) -> q c", c=RW)
                    Rbig = gth.tile([128, 8 * 33], BF16, tag=f"rbig{st}")
                    for tq in range(8):
                        q0 = tq * 128
                        lq = qT_sb[base:base + 64, b * L + q0: b * L + q0 + 128]
                        pR = psm.tile([128, 512], F32, tag="sm")
                        nc.tensor.matmul(pR[:, 0:33], lhsT=lq,
                                         rhs=relT_rep[base:base + 64, :],
                                         start=True, stop=True)
                        R0 = sml.tile([128, 1], F32, tag="r0")
                        nc.scalar.copy(out=R0, in_=pR[:, 0:1])
                        nc.vector.tensor_scalar_sub(Rbig[:, tq * 33:(tq + 1) * 33],
                                                    in0=pR[:, 0:33], scalar1=R0)
                    nc.scalar.dma_start(
                        out=bass.AP(tensor=rp.ap().tensor, offset=127,
                                    ap=[[RW, 128], [RW * 128, 8], [1, 33]]),
                        in_=Rbig)

                def emit_tile(b, hh, rp, ab, a, tq, attnT, ATd2_sb):
                    st = 2 * b + hh
                    base = 64 * hh
                    q0 = tq * 128
                    K = q0 + 128
                    lq = qT_sb[base:base + 64, b * L + q0: b * L + q0 + 128]
                    attn = at_pool.tile([128, 1024], BF16, tag=f"at{st}")
                    kf = max(0, q0 - 32)
                    # band gather once per tile
                    wb = K - kf
                    rbg = gth.tile([128, 160], BF16, tag=f"rbg{st}")
                    diag = bass.AP(tensor=rp.ap().tensor,
                                   offset=q0 * RW + 159 + (kf - q0),
                                   ap=[[RW - 1, 128], [1, wb]])
                    nc.scalar.dma_start(out=rbg[:, :wb], in_=diag)
                    dens = []
                    for (s, e) in pieces_of(tq):
                        ps = scps.tile([128, 512], F32, tag="sc")
                        nc.tensor.matmul(
                            ps[:, :e - s], lhsT=lq,
                            rhs=kT_sb[base:base + 64, b * L + s: b * L + e],
                            start=True, stop=True)
                        # add the part of the relative band overlapping this piece
                        lo, hi = max(s, kf), e
                        if lo < hi:
                            nc.vector.tensor_add(ps[:, lo - s:hi - s],
                                                 ps[:, lo - s:hi - s],
                                                 rbg[:, lo - kf:hi - kf])
                        # softmax without max-subtraction: post-scale scores are
                        # bounded (|s|/8 < ~4); masked lanes are -1e18 -> exp = 0.
                        den = sml.tile([128, 1], F32, tag="dn")
                        nc.scalar.activation(out=attn[:, s:e], in_=ps[:, :e - s],
                                             func=mybir.ActivationFunctionType.Exp,
                                             bias=0.0, scale=0.125, accum_out=den)
                        dens.append(den)
                    # unnormalized attn band -> ab (diagonal store), ASAP
                    k0a = max(0, q0 - 31)
                    wa = q0 + 128 - k0a
                    abdst = bass.AP(tensor=ab.ap().tensor,
                                    offset=q0 * AW + 31 + (k0a - q0),
                                    ap=[[AW - 1, 128], [1, wa]])
                    nc.sync.dma_start(out=abdst, in_=attn[:, k0a:k0a + wa])
                    if len(dens) > 1:
                        dtot = sml.tile([128, 1], F32, tag="dn")
                        nc.vector.tensor_add(dtot, dens[0], dens[1])
                    else:
                        dtot = dens[0]
                    rden = sml.tile([128, 1], F32, tag="rd")
                    nc.vector.reciprocal(rden, dtot)
                    # normalization folded into the transpose matmuls: rhs is
                    # diag(1/den) instead of the identity
                    dg = gth.tile([128, 128], BF16, tag=f"dg{st}")
                    nc.vector.tensor_scalar_mul(dg, in0=ident, scalar1=rden)
                    half = (tq - a) * 128
                    for kt in range(tq + 1):
                        ptr = psm.tile([128, 512], F32, tag="sm")
                        nc.tensor.matmul(ptr[:, 0:128],
                                         lhsT=attn[:, kt * 128:(kt + 1) * 128],
                                         rhs=dg, start=True, stop=True)
                        nc.vector.tensor_copy(
                            attnT[:, kt * 256 + half: kt * 256 + half + 128],
                            ptr[:, 0:128])
                    # attn diag band back (rows of ab, contiguous inner):
                    # Ad2[:, 0:64] = attn_u[q, q+j'-31]; col 64 = den - rowsum;
                    # then the whole [128, 65] row-block is scaled by 1/den.
                    Ad2 = gth.tile([128, 65], BF16, tag=f"ad2{st}")
                    nc.sync.dma_start(out=Ad2[:, 0:64], in_=bass.AP(
                        tensor=ab.ap().tensor, offset=q0 * AW,
                        ap=[[AW, 128], [1, 64]]))
                    Ssum = sml.tile([128, 1], F32, tag="ss")
                    nc.vector.reduce_sum(Ssum, Ad2[:, 0:64], axis=mybir.AxisListType.X)
                    nc.vector.tensor_sub(Ad2[:, 64:65], dtot, Ssum)
                    nc.vector.tensor_scalar_mul(Ad2, in0=Ad2, scalar1=rden)
                    ptr2 = psm.tile([128, 512], BF16, tag="sm")
                    nc.tensor.transpose(ptr2[0:65, 0:128], Ad2, ident)
                    nc.vector.tensor_copy(ATd2_sb[:, half:half + 128],
                                          ptr2[0:65, 0:128])

                def emit_pair(b, hh, rp, ab, a):
                    st = 2 * b + hh
                    base = 64 * hh
                    attnT = atT_pool.tile([128, 8 * 256], BF16)
                    ATd2_sb = gth.tile([65, 256], BF16, tag=f"atd{st}")
                    for tq in (a, a + 1):
                        emit_tile(b, hh, rp, ab, a, tq, attnT, ATd2_sb)
                    pctx = psc.tile([128, 256], F32)
                    rows = slice(base, base + 64)
                    for kt in range(a + 1):
                        nc.tensor.matmul(
                            pctx[rows, :],
                            lhsT=v_sb[:, (8 * b + kt) * 128 + base:
                                      (8 * b + kt) * 128 + base + 64],
                            rhs=attnT[:, kt * 256:(kt + 1) * 256],
                            start=(kt == 0), stop=False)
                    # k-tile a+1 only feeds q-tile a+1 (right half); skip the zero half
                    nc.tensor.matmul(
                        pctx[rows, 128:256],
                        lhsT=v_sb[:, (8 * b + a + 1) * 128 + base:
                                  (8 * b + a + 1) * 128 + base + 64],
                        rhs=attnT[:, (a + 1) * 256 + 128:(a + 2) * 256],
                        start=False, stop=False)
                    nc.tensor.matmul(pctx[rows, :], lhsT=relvp_sb, rhs=ATd2_sb,
                                     start=False, stop=True)
                    nc.scalar.copy(out=ctxT_sb[rows, b * L + a * 128: b * L + a * 128 + 256],
                                   in_=pctx[rows, :])

                # all four (batch, head) streams interleaved to keep the PE
                # dense; output projection of each finished 512-token column
                # range interleaved to fill PE gaps
                for b in range(2):
                    for hh in (0, 1):
                        emit_phaseA(b, hh, rpads[2 * b + hh])
                for a in (0, 2, 4, 6):
                    for b in range(2):
                        for hh in (0, 1):
                            emit_pair(b, hh, rpads[2 * b + hh], abufs[2 * b + hh], a)
                    if a == 4:
                        out_proj(0, 0, 8)
                        out_proj(2, 0, 8)
                out_proj(1, 0, 8)
                out_proj(3, 0, 8)
    nc.compile()
    return nc


_PERM64 = np.concatenate([np.arange(0, 64, 2), np.arange(1, 64, 2)])


def prep_inputs(x, w_q, w_k, w_v, w_o, rel_emb):
    """Host-side sharding/layout prep. Returns per-core input maps."""
    bf = ml_dtypes.bfloat16
    xT = np.ascontiguousarray(x.reshape(T, MD).T).astype(bf)
    relT33 = np.ascontiguousarray(rel_emb.T[_PERM64][:, 0:33]).astype(bf)
    permv = np.concatenate([np.arange(1, 65), [0]])
    relvp = np.ascontiguousarray(rel_emb[permv, :]).astype(bf)
    in_maps = []
    for c in range(NCORES):
        rows = np.arange(128 * c, 128 * (c + 1))
        rows_d = np.concatenate([128 * c + 64 * h + _PERM64 for h in range(HC)])
        in_maps.append({
            "xT": xT,
            "wqT": np.ascontiguousarray(w_q[rows_d, :].T).astype(bf),
            "wkT": np.ascontiguousarray(w_k[rows_d, :].T).astype(bf),
            "wvT": np.ascontiguousarray(w_v[rows, :].T).astype(bf),
            "woT": np.ascontiguousarray(w_o[:, rows].T).astype(bf),
            "relT33": relT33,
            "relvp": relvp,
        })
    return in_maps


_NC_CACHE = None


def get_nc():
    global _NC_CACHE
    if _NC_CACHE is None:
        _NC_CACHE = build_nc()
    return _NC_CACHE


def kernel(x, w_q, w_k, w_v, w_o, rel_emb):
    x = np.asarray(x, dtype=np.float32)
    in_maps = prep_inputs(np.asarray(x, np.float32), np.asarray(w_q, np.float32),
                          np.asarray(w_k, np.float32), np.asarray(w_v, np.float32),
                          np.asarray(w_o, np.float32), np.asarray(rel_emb, np.float32))
    nc = get_nc()
    res = run_bass_kernel_spmd(nc, in_maps, list(range(NCORES)))
    acc = np.zeros((MD, T), dtype=np.float32)
    for r in res.results:
        acc += r["outT"]
    return np.ascontiguousarray(acc.T).reshape(B, L, MD)
